# revision 31
# baseline (speedup 1.0000x reference)
"""Kitsune (ensemble of tiny autoencoders) Bass kernel for Trainium2, 8 NeuronCores.

Strategy (pure data parallel, batch sharded 8 ways).

The wall-clock is dominated by the axon tunnel (~75 MB/s H2D, ~49 MB/s D2H,
not duplex, ~85 ms/NEFF execute overhead) and a single-CPU host, so the
design minimizes wire bytes + host passes and pipelines what it can:
  - x is quantized on host to uint8 (52.5 MB instead of 210 MB f32); the
    dequant scale (u+0.5)/255 is folded into the on-chip normalise constants
    and encoder weights.  Quantization error <=1/510 on x.
  - Both outputs are quantized on-device to u8, written row-major as two
    separate [R, 10] tensors.  D2H is 10.5 MB total, and because both async
    fetches are pre-queued before the NEFF finishes, the terminal pushes
    them back-to-back with no request round-trip: t's dequant multiply runs
    while x_hat is still streaming, making D2H nearly free (~0.08 s vs
    0.25 s for a single post-hoc fetch).  The DVE f32->u8 convert rounds to
    nearest, so the host dequant offset is 0.
  - N_CHUNKS sequential smaller NEFF calls were measured a wash (tunnel is
    not duplex; hidden execute == extra dispatch), so N_CHUNKS=1.
  - The jax shard_map jit is built once and cached; replicated parameters
    are device-cached keyed on their content; output operand buffers are
    persistent device zeros (no donation, no per-call zeros upload).
  - Build functions get a fixed pseudo-filename so the BIR (which embeds
    caller file/line debug info) is byte-identical regardless of where
    kernel.py lives -> the neuron compile cache hits across directories.

On-chip (per core, per chunk):
  - fp16 compute, fp32 PSUM accumulation.
  - u8 rows are DMA-cast u8->f16 batch-major into padded [CH,128] DRAM
    staging, then transposed to feature-major [100, batch] via xbar DMA
    transpose.
  - All 10 tail autoencoders run as single block-diagonal matmuls
    (enc [100,80], dec [80,100]); input normalisation folded into enc
    weights host-side.
  - Per-cluster RMSE reduce = block-diagonal matmul with 0.1-weighted
    cluster-membership stationary; 12 consecutive 512-row blocks accumulate
    into one PSUM tile [120, 512] at partition offsets 10k.
  - sqrt lives in a different ACT table set than sigmoid, so phase 2
    (sqrt of all losses, then head AE + u8 outputs) runs after phase 1.
"""
import os
import sys
import time
import types
sys.path.insert(0, '/opt/trn_rl_repo')

import numpy as np

import concourse.bass as bass
import concourse.bacc as bacc
import concourse.tile as tile
import concourse.mybir as mybir

dt = mybir.dt
A = mybir.AluOpType
ACTF = mybir.ActivationFunctionType

N_CORES = 8
# chunked pipelining was measured a wash: the tunnel is not duplex, so
# splitting only trades hidden execute overhead for extra D2H latency +
# dispatch; single-call is the optimal schedule.
N_CHUNKS = 1
B = 524288
C, F, H, HC = 10, 10, 8, 8
D = C * F              # 100
EH = C * H             # 80
BROWS = B // N_CHUNKS  # rows per chunk (global)
RC = BROWS // N_CORES  # rows per core per chunk
BS = 512               # rows per block
CH = 4096              # rows per staging chunk
EPS = np.float32(1e-16)
Q = np.float32(1.0 / 255.0)   # u8 dequant scale
_DEQ_OFS = 0.0                # output dequant offset: 0.0 because the DVE
                              # f->u8 convert rounds to nearest (measured)

_cached = {}


def _groups(nblk):
    q, r = divmod(nblk, 12)
    return [12] * q + ([r] if r else [])


def _build_module(rows):
    nc = bacc.Bacc(None, target_bir_lowering=False, debug=False,
                   num_devices=N_CORES)
    x_d = nc.dram_tensor("x", [rows, D], dt.uint8, kind="ExternalInput")
    enc_w_d = nc.dram_tensor("enc_w", [D, EH], dt.float16, kind="ExternalInput")
    dec_w_d = nc.dram_tensor("dec_w", [EH, D], dt.float16, kind="ExternalInput")
    red_w_d = nc.dram_tensor("red_w", [D, 120 * 12], dt.float16, kind="ExternalInput")
    he_w_d = nc.dram_tensor("he_w", [120, 96], dt.float16, kind="ExternalInput")
    hd_w_d = nc.dram_tensor("hd_w", [96, 120], dt.float16, kind="ExternalInput")
    vecs_d = nc.dram_tensor("vecs", [128, 10], dt.float32, kind="ExternalInput")
    # four separate outputs (row-halves of t and x_hat): the pre-queued async
    # fetches stream back-to-back, and the host dequants each piece while the
    # next is still on the wire
    half = rows // 2
    tqa_d = nc.dram_tensor("t_qa", [half, C], dt.uint8, kind="ExternalOutput")
    tqb_d = nc.dram_tensor("t_qb", [half, C], dt.uint8, kind="ExternalOutput")
    xqa_d = nc.dram_tensor("xh_qa", [half, C], dt.uint8, kind="ExternalOutput")
    xqb_d = nc.dram_tensor("xh_qb", [half, C], dt.uint8, kind="ExternalOutput")

    with tile.TileContext(nc) as tc:
        _kernel_body(nc, tc, rows, x_d, enc_w_d, dec_w_d, red_w_d, he_w_d,
                     hd_w_d, vecs_d, (tqa_d, tqb_d), (xqa_d, xqb_d))
    nc.finalize()
    return nc


def _kernel_body(nc, tc, rows, x_d, enc_w_d, dec_w_d, red_w_d, he_w_d, hd_w_d,
                 vecs_d, tq_ds, xq_ds):
    from contextlib import ExitStack
    groups = _groups(rows // BS)
    with ExitStack() as ctx:
        const = ctx.enter_context(tc.tile_pool(name="const", bufs=1))
        # -- load parameters once --
        enc_w = const.tile([D, EH], dt.float16)
        nc.sync.dma_start(enc_w[:], enc_w_d.ap())
        dec_w = const.tile([EH, D], dt.float16)
        nc.sync.dma_start(dec_w[:], dec_w_d.ap())
        red_w = const.tile([D, 120 * 12], dt.float16)
        nc.sync.dma_start(red_w[:], red_w_d.ap())
        he_w = const.tile([120, 96], dt.float16)
        nc.sync.dma_start(he_w[:], he_w_d.ap())
        hd_w = const.tile([96, 120], dt.float16)
        nc.sync.dma_start(hd_w[:], hd_w_d.ap())
        vecs = const.tile([128, 10], dt.float32)
        nc.sync.dma_start(vecs[:], vecs_d.ap())
        # vecs columns: 0=a_vec[100], 1=c_vec[100], 2=enc_b[80], 3=dec_b[100],
        #               4=he_b[96], 5=hd_b[120], 6=255*at_vec[120],
        #               7=255*ct_vec[120], 8=255.0[120], 9=0.0[120]
        a_v = vecs[0:D, 0:1]
        c_v = vecs[0:D, 1:2]
        enc_b = vecs[0:EH, 2:3]
        dec_b = vecs[0:D, 3:4]
        he_b = vecs[0:96, 4:5]
        hd_b = vecs[0:120, 5:6]
        at_v = vecs[0:120, 6:7]
        ct_v = vecs[0:120, 7:8]
        s255_v = vecs[0:120, 8:9]
        zofs_v = vecs[0:120, 9:10]

        Spool = ctx.enter_context(tc.tile_pool(name="Sbuf", bufs=len(groups)))
        S_tiles = []

        # ---------------- phase 1 ----------------
        with tc.tile_pool(name="st", bufs=rows // CH, space="DRAM") as stp, \
             tc.tile_pool(name="xt", bufs=3) as xtp, \
             tc.tile_pool(name="act", bufs=2) as actp, \
             tc.tile_pool(name="z1", bufs=2, space="PSUM") as z1p, \
             tc.tile_pool(name="z2", bufs=1, space="PSUM") as z2p, \
             tc.tile_pool(name="Sps", bufs=2, space="PSUM") as Sp:
            # stage x as padded f16 rows [CH, 128] via fat DRAM->DRAM cast DMAs
            # (u8 -> f16: values 0..255, exactly representable)
            x16_tiles = []
            for ch in range(rows // CH):
                x16 = stp.tile([CH, 128], dt.float16, tag="x16")
                nc.gpsimd.dma_start(x16[:, 0:D], x_d.ap()[ch * CH:(ch + 1) * CH, :])
                x16_tiles.append(x16)
            blk = 0
            for g, gn in enumerate(groups):
                S_ps = Sp.tile([120, BS], dt.float32, tag="Sps")
                for pp in range(gn // 2):
                    kk = 2 * pp
                    rows0 = blk * BS
                    # one DRAM->SBUF xbar transpose for the whole 1024-row pair
                    ch, off = rows0 // CH, rows0 % CH
                    xt = xtp.tile([128, 2 * BS], dt.float16, tag="xt")
                    nc.sync.dma_start(xt[:], x16_tiles[ch][off:off + 2 * BS, :],
                                      transpose=True)
                    # xn for the loss (normalise + dequant folded: a*u + c)
                    xn = actp.tile([D, 2 * BS], dt.float16, tag="xn")
                    nc.vector.tensor_scalar(xn[:], xt[0:D, :], a_v, c_v,
                                            A.mult, A.add)
                    # encoder (normalise + dequant folded into weights) + sigmoid
                    z1 = z1p.tile([EH, 2 * BS], dt.float32, tag="z1")
                    nc.tensor.matmul(z1[:, 0:BS], enc_w[:], xt[0:D, 0:BS],
                                     start=True, stop=True)
                    nc.tensor.matmul(z1[:, BS:2 * BS], enc_w[:], xt[0:D, BS:2 * BS],
                                     start=True, stop=True)
                    h = actp.tile([EH, 2 * BS], dt.float16, tag="h")
                    nc.scalar.activation(h[:], z1[:], ACTF.Sigmoid, bias=enc_b)
                    # decoder + sigmoid
                    z2 = z2p.tile([D, 2 * BS], dt.float32, tag="z2")
                    nc.tensor.matmul(z2[:, 0:BS], dec_w[:], h[:, 0:BS],
                                     start=True, stop=True)
                    nc.tensor.matmul(z2[:, BS:2 * BS], dec_w[:], h[:, BS:2 * BS],
                                     start=True, stop=True)
                    rec = actp.tile([D, 2 * BS], dt.float16, tag="rec")
                    nc.scalar.activation(rec[:], z2[:], ACTF.Sigmoid, bias=dec_b)
                    # squared error
                    dd = actp.tile([D, 2 * BS], dt.float16, tag="dd")
                    nc.vector.tensor_tensor(dd[:], rec[:], xn[:], A.subtract)
                    sq = actp.tile([D, 2 * BS], dt.float16, tag="sq")
                    nc.vector.tensor_tensor(sq[:], dd[:], dd[:], A.mult)
                    # per-cluster mean reduce, stacked at partition 10*slot
                    nc.tensor.matmul(S_ps[:], red_w[:, 120 * kk:120 * (kk + 1)],
                                     sq[:, 0:BS], start=(kk == 0), stop=False,
                                     skip_group_check=True)
                    nc.tensor.matmul(S_ps[:], red_w[:, 120 * (kk + 1):120 * (kk + 2)],
                                     sq[:, BS:2 * BS], start=False,
                                     stop=(kk + 1 == gn - 1), skip_group_check=True)
                    blk += 2
                S_sb = Spool.tile([120, BS], dt.float32, tag="Ssb")
                nc.scalar.activation(S_sb[0:10 * gn, :], S_ps[0:10 * gn, :],
                                     ACTF.Copy)
                S_tiles.append(S_sb)

        # ---------------- phase 2 ----------------
        with tc.tile_pool(name="tails", bufs=len(groups)) as tailp, \
             tc.tile_pool(name="hact", bufs=2) as hactp, \
             tc.tile_pool(name="z3", bufs=2, space="PSUM") as z3p, \
             tc.tile_pool(name="z4", bufs=2, space="PSUM") as z4p:
            tails_tiles = []
            for g, gn in enumerate(groups):
                P10 = 10 * gn
                tl = tailp.tile([120, BS], dt.float16, tag="tails")
                nc.scalar.activation(tl[0:P10, :], S_tiles[g][0:P10, :], ACTF.Sqrt)
                tails_tiles.append(tl)
            for g, gn in enumerate(groups):
                P10, P8 = 10 * gn, 8 * gn
                tl = tails_tiles[g]
                z3 = z3p.tile([96, BS], dt.float32, tag="z3")
                nc.tensor.matmul(z3[0:P8, :], he_w[0:P10, 0:P8], tl[0:P10, :],
                                 start=True, stop=True)
                hh = hactp.tile([96, BS], dt.float16, tag="hh")
                nc.scalar.activation(hh[0:P8, :], z3[0:P8, :], ACTF.Sigmoid,
                                     bias=he_b[0:P8, :])
                z4 = z4p.tile([120, BS], dt.float32, tag="z4")
                nc.tensor.matmul(z4[0:P10, :], hd_w[0:P8, 0:P10], hh[0:P8, :],
                                 start=True, stop=True)
                xh = hactp.tile([120, BS], dt.float16, tag="xh")
                nc.scalar.activation(xh[0:P10, :], z4[0:P10, :], ACTF.Sigmoid,
                                     bias=hd_b[0:P10, :])
                # quantize both outputs to u8 (scale 255 folded into at/ct)
                tv = hactp.tile([120, BS], dt.uint8, tag="tv")
                nc.vector.tensor_scalar(tv[0:P10, :], tl[0:P10, :],
                                        at_v[0:P10, :], ct_v[0:P10, :],
                                        A.mult, A.add)
                xq = hactp.tile([120, BS], dt.uint8, tag="xq")
                nc.vector.tensor_scalar(xq[0:P10, :], xh[0:P10, :],
                                        s255_v[0:P10, :], zofs_v[0:P10, :],
                                        A.mult, A.add)
                # scatter to row-major [rows/2, 10] halves: row = rowbase+512*k+j
                rowbase = 12 * BS * g
                half = rows // 2
                for k in range(gn):
                    r0 = rowbase + BS * k
                    hi, ro = (1, r0 - half) if r0 >= half else (0, r0)
                    t_ap = tq_ds[hi].ap()[ro:ro + BS, :].rearrange("j c -> c j")
                    nc.sync.dma_start(t_ap, tv[10 * k:10 * k + 10, :])
                    x_ap = xq_ds[hi].ap()[ro:ro + BS, :].rearrange("j c -> c j")
                    nc.scalar.dma_start(x_ap, xq[10 * k:10 * k + 10, :])


def _stable_filename(fn, fname="<kitsune_bass_kernel>"):
    """Rebind fn to a code object with a fixed co_filename so the BIR's
    ant_debug info (and hence the neuron compile-cache key) does not depend
    on where this file lives."""
    def fix(code):
        consts = tuple(fix(c) if isinstance(c, types.CodeType) else c
                       for c in code.co_consts)
        return code.replace(co_consts=consts, co_filename=fname)
    return types.FunctionType(fix(fn.__code__), fn.__globals__, fn.__name__,
                              fn.__defaults__, fn.__closure__)


_kernel_body = _stable_filename(_kernel_body)
_build_module = _stable_filename(_build_module)


def _fold_params(i):
    """Host-side folding of all small parameters. i = inputs dict (np f32).

    x arrives as u8 with dequant x ~= (u+0.5)/255, folded in here.
    """
    aw = 1.0 / (i["tail_nmax"].astype(np.float32) - i["tail_nmin"] + EPS)  # [C,F]
    cw = -i["tail_nmin"] * aw
    We = i["tail_enc_w"].astype(np.float32)       # [C,H,F]
    be = i["tail_enc_b"].astype(np.float32)       # [C,H]
    Wef = We * aw[:, None, :]
    bef = be + np.einsum('chf,cf->ch', We, cw)
    # u8 dequant fold: z = Wef @ ((u+0.5)/255) + bef
    bef = bef + 0.5 * Q * Wef.sum(axis=2)
    Wef = Wef * Q
    enc_w = np.zeros((D, EH), np.float16)
    dec_w = np.zeros((EH, D), np.float16)
    Wd = i["tail_dec_w"].astype(np.float32)       # [C,F,H]
    for c in range(C):
        enc_w[10 * c:10 * c + F, 8 * c:8 * c + H] = Wef[c].T  # [F,H]
        dec_w[8 * c:8 * c + H, 10 * c:10 * c + F] = Wd[c].T   # [H,F]
    red_w = np.zeros((D, 120 * 12), np.float16)
    for k in range(12):
        for c in range(C):
            red_w[10 * c:10 * c + F, 120 * k + 10 * k + c] = 0.1
    at = 1.0 / (i["head_nmax"].astype(np.float32) - i["head_nmin"] + EPS)  # [10]
    ct = -i["head_nmin"] * at
    Whe = i["head_enc_w"].astype(np.float32)      # [HC, C]
    bhe = i["head_enc_b"].astype(np.float32) + Whe @ ct
    Whef = Whe * at[None, :]
    Whd = i["head_dec_w"].astype(np.float32)      # [C, HC]
    bhd = i["head_dec_b"].astype(np.float32)
    he_w = np.zeros((120, 96), np.float16)
    hd_w = np.zeros((96, 120), np.float16)
    for k in range(12):
        he_w[10 * k:10 * k + C, 8 * k:8 * k + HC] = Whef.T
        hd_w[8 * k:8 * k + HC, 10 * k:10 * k + C] = Whd.T
    vecs = np.zeros((128, 10), np.float32)
    # xn = aw*x + cw with x ~= (u+0.5)/255  ->  (aw/255)*u + (cw + 0.5*aw/255)
    vecs[0:D, 0] = (aw * Q).reshape(-1)
    vecs[0:D, 1] = (cw + 0.5 * Q * aw).reshape(-1)
    vecs[0:EH, 2] = bef.reshape(-1)
    vecs[0:D, 3] = i["tail_dec_b"].astype(np.float32).reshape(-1)
    vecs[0:96, 4] = np.tile(bhe, 12)
    vecs[0:120, 5] = np.tile(bhd, 12)
    # u8 output quantization: u = convert(255*t) (scale folded into at/ct)
    vecs[0:120, 6] = np.tile(at * 255.0, 12)
    vecs[0:120, 7] = np.tile(ct * 255.0, 12)
    vecs[0:120, 8] = 255.0
    vecs[0:120, 9] = 0.0
    return dict(enc_w=enc_w, dec_w=dec_w, red_w=red_w, he_w=he_w, hd_w=hd_w,
                vecs=vecs)


PARAM_NAMES = ("tail_enc_w", "tail_enc_b", "tail_dec_w", "tail_dec_b",
               "tail_nmin", "tail_nmax", "head_enc_w", "head_enc_b",
               "head_dec_w", "head_dec_b", "head_nmin", "head_nmax")


def _get_runner():
    """Build the Bass module and a cached shard_map jit around it (once)."""
    if "runner" in _cached:
        return _cached["runner"]

    import jax
    import jax.numpy as jnp
    from jax.sharding import Mesh, PartitionSpec, NamedSharding
    from jax.experimental.shard_map import shard_map
    from concourse.bass2jax import (_bass_exec_p, install_neuronx_cc_hook,
                                    partition_id_tensor)

    install_neuronx_cc_hook()
    nc = _build_module(RC)

    partition_name = nc.partition_id_tensor.name if nc.partition_id_tensor else None
    in_names, out_names, out_avals = [], [], []
    for alloc in nc.m.functions[0].allocations:
        if not isinstance(alloc, mybir.MemoryLocationSet):
            continue
        name = alloc.memorylocations[0].name
        if alloc.kind == "ExternalInput":
            if name != partition_name:
                in_names.append(name)
        elif alloc.kind == "ExternalOutput":
            out_names.append(name)
            out_avals.append(jax.core.ShapedArray(
                tuple(alloc.tensor_shape), mybir.dt.np(alloc.dtype)))
    n_params = len(in_names)
    n_outs = len(out_avals)
    all_in_names = in_names + out_names
    if partition_name is not None:
        all_in_names.append(partition_name)

    def _body(*args):
        operands = list(args)
        if partition_name is not None:
            operands.append(partition_id_tensor())
        outs = _bass_exec_p.bind(
            *operands,
            out_avals=tuple(out_avals),
            in_names=tuple(all_in_names),
            out_names=tuple(out_names),
            lowering_input_output_aliases=(),
            sim_require_finite=True,
            sim_require_nnan=True,
            nc=nc,
        )
        return tuple(outs)

    devices = jax.devices()[:N_CORES]
    mesh = Mesh(np.asarray(devices), ("core",))
    sharding = NamedSharding(mesh, PartitionSpec("core"))
    in_specs = (PartitionSpec("core"),) * (n_params + n_outs)
    out_specs = (PartitionSpec("core"),) * n_outs
    # No donation: the kernel writes every output element, so the output
    # operand buffers are just placeholders — keep persistent device zeros
    # and reuse them every call (no per-call H2D of zeros).
    sharded = jax.jit(
        shard_map(_body, mesh=mesh, in_specs=in_specs, out_specs=out_specs,
                  check_rep=False),
        keep_unused=True)

    zeros_dev = [
        jax.jit(lambda a=a: jnp.zeros((N_CORES * a.shape[0], *a.shape[1:]),
                                      a.dtype), out_shardings=sharding)()
        for a in out_avals
    ]
    for z in zeros_dev:
        z.block_until_ready()

    x_u8 = np.empty((B, D), np.uint8)
    state = dict(jax=jax, sharding=sharding, sharded=sharded,
                 in_names=in_names, out_names=out_names,
                 zeros_dev=zeros_dev, x_u8=x_u8, param_key=None,
                 param_dev=None)
    _cached["runner"] = state
    return state


# The jit-level HLO embeds source locations of the traced functions (_body,
# the zeros lambda), which feed the neuron compile-cache fingerprint — fix
# their co_filename too so the cache hits from any directory.
_get_runner = _stable_filename(_get_runner)


def kernel(**inputs):
    prof = os.environ.get("BASSK_PROF")
    tmarks = [time.perf_counter()]
    def mark(label):
        if prof:
            tmarks.append(time.perf_counter())
            print(f"    [{label}] {(tmarks[-1]-tmarks[-2])*1e3:.0f} ms", flush=True)
    def _heal():
        # transient device wedge (NRT_EXEC_UNIT_UNRECOVERABLE / mesh desynced
        # at executable launch while raw transfers work): wait, then probe
        # each core — it self-recovers; then retry once.
        import jax as _jax
        time.sleep(40)
        for d in _jax.devices()[:N_CORES]:
            _jax.device_put(np.zeros((4,), np.float32), d).block_until_ready()

    try:
        st = _get_runner()
    except Exception:
        _heal()
        st = _get_runner()
    jax = st["jax"]
    inputs = {k: np.asarray(v) for k, v in inputs.items()}
    mark("setup")

    x = inputs["x"]

    # replicated small parameters: fold + device-put once per distinct values
    key = b"".join(np.ascontiguousarray(inputs[n]).tobytes()
                   for n in PARAM_NAMES)
    if st["param_key"] != key:
        params = _fold_params(inputs)
        dev = {}
        for n in st["in_names"]:
            if n == "x":
                continue
            rep = np.concatenate([params[n]] * N_CORES, axis=0)
            dev[n] = jax.device_put(rep, st["sharding"])
        st["param_dev"] = dev
        st["param_key"] = key
    mark("params")

    onames = ("t_qa", "t_qb", "xh_qa", "xh_qb")
    oidx = [st["out_names"].index(n) for n in onames]
    HALF = RC // 2

    def _execute():
        # pipeline: quantize + dispatch each chunk; the next chunk's quantize
        # and dispatch overlap the previous chunk's H2D/execute on the tunnel
        outs_async = []
        for cidx in range(N_CHUNKS):
            r0 = cidx * BROWS
            xu = st["x_u8"][r0:r0 + BROWS]
            # quantize x to u8 (truncation; device dequants as (u+0.5)/255)
            np.multiply(x[r0:r0 + BROWS], 255., out=xu, casting='unsafe')
            args = [xu if n == "x" else st["param_dev"][n]
                    for n in st["in_names"]] + st["zeros_dev"]
            out_arrs = st["sharded"](*args)
            # pre-queue all D2H fetches: they stream back-to-back the moment
            # the NEFF finishes, with no per-request round-trip
            for i in oidx:
                out_arrs[i].copy_to_host_async()
            outs_async.append(out_arrs)
            mark(f"chunk{cidx} quant+dispatch")

        # dequant: value = (u + ofs)/255; ofs=0, the DVE rounds to nearest.
        # Each piece's dequant runs while the next piece is still on the wire.
        # Global piece row k*HALF+j maps to original row k*RC + h*HALF + j.
        # Allocate + prefault the output pages now, inside the H2D/exec wait
        # window, so the dequant multiplies don't pay page faults in the tail.
        t_out = np.empty((B, C), np.float32)
        x_hat = np.empty((B, C), np.float32)
        t_out.reshape(-1)[::1024] = 0.0
        x_hat.reshape(-1)[::1024] = 0.0
        dsts = ((t_out, 0), (t_out, 1), (x_hat, 0), (x_hat, 1))
        for cidx in range(N_CHUNKS):
            base = cidx * BROWS
            for i, (dst, h) in zip(oidx, dsts):
                u8 = np.asarray(outs_async[cidx][i])
                mark(f"chunk{cidx} D2H {st['out_names'][i]}")
                for k in range(N_CORES):
                    d0 = base + k * RC + h * HALF
                    np.multiply(u8[k * HALF:(k + 1) * HALF], Q,
                                out=dst[d0:d0 + HALF])
                    if _DEQ_OFS:
                        dst[d0:d0 + HALF] += np.float32(_DEQ_OFS * Q)
                mark(f"chunk{cidx} dequant {st['out_names'][i]}")
        return x_hat, t_out

    try:
        return _execute()
    except Exception:
        _heal()
        return _execute()
